# revision 51
# baseline (speedup 1.0000x reference)
"""Trainium2 Bass kernel for nn_Attention_79164837199973 (v7).

Bias-augmented multi-head self-attention with sigmoid gating.
B=4, N=1024, CQ=CH=512, H=8, D=64.

Sharding (8 cores, no collectives): core c -> batch b=c//2, query-row half
r=c%2 (512 rows). Keys are PERMUTED per core (own query-row half first) so
the q-projection reads a prefix slice of the same x^T tile -- no separate
xtq DMA. kt/ebt/vaug all use the permuted key order (softmax sum order is
irrelevant).

v7 changes vs v6 (trace-driven):
  - HAM warmth: the v6 schedule had >3.4us PE-idle windows; the PE spent
    ~38us re-throttled at 1.2 GHz (MM 630ns vs 379 warm). v7 interleaves
    projection/PV/gate/to_out matmuls as fillers inside the exp-paced
    attention pairs so the PE never idles long enough to re-throttle.
  - engine balance: ACT = exps + qt0 evac + oraw evacs + tanh-gate
    (tanh shares the exp table set; sigmoid does NOT -- gate =
    0.5*tanh(0.5 z + 0.5 gb)+0.5). DVE = kt/v/qt1-3 evacs + bias-mults +
    norm chain + to_out fusion.
  - norm: reciprocal_approx_fast on f32 (0.67us vs 3.3us DVE reciprocal),
    rbc broadcast matmul result multiplied straight from PSUM (kills the
    ACT copy). Norm in 3 batches (h0-3 mid, h4-5 late-mid, h6-7 tail).
  - to_out: all 4 ko accumulate in ONE PSUM bank per mo at the tail, then
    a single scalar_tensor_tensor (ps + bo) * gate evacuation.
  - ebt DMA in 16 chunks (pair,t)-ordered for arrival pacing.
"""

import os
import sys

sys.path.insert(0, "/opt/trn_rl_repo")

import numpy as np

import concourse.bass as bass
import concourse.tile as tile
from concourse import bacc, mybir

B, N, CQ, CH, H = 4, 1024, 512, 512, 8
D = CH // H  # 64
NQ = N // 2  # 512 query rows per core
P = 128
F32 = mybir.dt.float32
BF16 = mybir.dt.bfloat16
AF = mybir.ActivationFunctionType
OP = mybir.AluOpType


def build_nc():
    nc = bacc.Bacc("TRN2", target_bir_lowering=False, debug=False, num_devices=8)

    # ---- DRAM parameters, already in SBUF layout (host pre-swizzled) ----
    # kt/qt/gate projections run fp8e4 DoubleRow (weights host-scaled by 32
    # against denormals, descaled in the evacuations); v stays bf16.
    FP8 = mybir.dt.float8e4
    xt_e = nc.declare_dram_parameter("xt", [P, 4, N], BF16, isOutput=False)
    xt8_e = nc.declare_dram_parameter("xt8", [P, 4, NQ], FP8, isOutput=False)
    ebt_e = nc.declare_dram_parameter("ebt", [P, H * 8, NQ], BF16, isOutput=False)
    wqt_e = nc.declare_dram_parameter("wqt", [P, 4, CH], BF16, isOutput=False)
    wkt_e = nc.declare_dram_parameter("wkt", [P, 4, CH], BF16, isOutput=False)
    wvt_e = nc.declare_dram_parameter("wvt", [P, 4, CH], BF16, isOutput=False)
    wot_e = nc.declare_dram_parameter("wot", [P, 4, CQ], BF16, isOutput=False)
    wot2_e = nc.declare_dram_parameter("wot2", [D, CQ], BF16, isOutput=False)
    wgt_e = nc.declare_dram_parameter("wgt", [P, 4, CQ], FP8, isOutput=False)
    bqs_e = nc.declare_dram_parameter("bqs", [P, 4], F32, isOutput=False)
    bo_e = nc.declare_dram_parameter("bo", [P, 4], F32, isOutput=False)
    gb_e = nc.declare_dram_parameter("gb", [P, 4], F32, isOutput=False)
    sel8_e = nc.declare_dram_parameter("sel8", [H, H * D], BF16, isOutput=False)
    out_e = nc.declare_dram_parameter("out", [P, 4, NQ], BF16, isOutput=True)

    with tile.TileContext(nc) as tc:
        with (
            tc.tile_pool(name="singles", bufs=1) as singles,
            tc.tile_pool(name="etmp", bufs=3) as etmp,
            tc.tile_pool(name="ps_s", bufs=2, space="PSUM") as ps_s,
            tc.tile_pool(name="ps_pv", bufs=2, space="PSUM") as ps_pv,
            tc.tile_pool(name="ps_pr", bufs=2, space="PSUM") as ps_pr,
        ):
            # ---- persistent SBUF tiles ----
            xt_sb = singles.tile([P, 4, N], BF16)
            xt8_sb = singles.tile([P, 4, NQ], FP8)
            wqt_sb = singles.tile([P, 4, CH], BF16)
            wkt_sb = singles.tile([P, 4, CH], BF16)
            wvt_sb = singles.tile([P, 4, CH], BF16)
            wot_sb = singles.tile([P, 4, CQ], BF16)
            wot2_sb = singles.tile([D, CQ], BF16)
            wgt_sb = singles.tile([P, 4, CQ], FP8)
            bqs_sb = singles.tile([P, 4], F32)
            bo_sb = singles.tile([P, 4], F32)
            gb_sb = singles.tile([P, 4], F32)
            sel8_sb = singles.tile([H, H * D], BF16)
            ebt_sb = singles.tile([P, H * 8, NQ], BF16)
            kt_sb = singles.tile([P, 4, N], BF16)
            qt_sb = singles.tile([P, 4, NQ], BF16)
            vaug_sb = singles.tile([P, 8, H * (D + 1)], BF16)
            oraw_sb = singles.tile([D + 1, H, NQ], BF16)  # 0-63 o, row 64 den
            osc_sb = singles.tile([D, H, NQ], BF16)  # normalized o
            ofin_sb = singles.tile([P, 4, NQ], BF16)  # head-merged o^T
            gate_sb = singles.tile([P, 4, NQ], BF16)
            outf_sb = singles.tile([P, 4, NQ], BF16)
            warm_sb = singles.tile([1, 8], F32)
            warmo_sb = singles.tile([1, 8], BF16)

            # force the exp table load off the critical path (first ACT op)
            nc.vector.memset(warm_sb, 0.0)
            nc.scalar.activation(out=warmo_sb, in_=warm_sb, func=AF.Exp)

            # PE pre-warm during the input-DMA wait: K must be 128 (the HAM
            # watches ARRAY activity -- a K=1 matmul engages 1 of 128 rows
            # and never un-throttles the clock), and N small so the stream
            # does not delay the first real matmuls.
            warm_big = singles.tile([P, 8], BF16)
            warmr_sb = singles.tile([P, P], BF16)
            nc.vector.memset(warm_big, 1.0)
            nc.vector.memset(warmr_sb, 1.0)
            warm_ps = ps_pr.tile([8, P], F32, tag="pr", name="warm_ps")
            for _ in range(24):
                nc.tensor.matmul(
                    warm_ps, lhsT=warm_big, rhs=warmr_sb, start=True, stop=True
                )

            # ---- input DMAs: FIFO order = priority order; mo0 slices of
            # wkt/wqt split out so kt(0,0)/qt(0) start ~4us earlier ----
            nc.sync.dma_start(out=wkt_sb[:, :, 0:P], in_=wkt_e[:, :, 0:P])
            # per-ko chunks: kt(0,0)'s accumulation starts on chunk 0 arrival
            for ko in range(4):
                nc.sync.dma_start(
                    out=xt_sb[:, ko, 0:NQ], in_=xt_e[:, ko, 0:NQ]
                )
            nc.sync.dma_start(out=wqt_sb[:, :, 0:P], in_=wqt_e[:, :, 0:P])
            nc.sync.dma_start(out=bqs_sb, in_=bqs_e[:, :])
            nc.sync.dma_start(out=wkt_sb[:, :, P:CH], in_=wkt_e[:, :, P:CH])
            nc.sync.dma_start(out=xt_sb[:, :, NQ:N], in_=xt_e[:, :, NQ:N])
            nc.sync.dma_start(out=wvt_sb, in_=wvt_e[:, :, :])
            nc.sync.dma_start(out=wqt_sb[:, :, P:CH], in_=wqt_e[:, :, P:CH])
            nc.sync.dma_start(out=xt8_sb, in_=xt8_e[:, :, :])

            def ebt_load(p, t):
                r0 = p * 16 + t * 4
                nc.sync.dma_start(
                    out=ebt_sb[:, r0 : r0 + 4, :], in_=ebt_e[:, r0 : r0 + 4, :]
                )

            for t in range(4):
                ebt_load(0, t)
            nc.sync.dma_start(out=gb_sb, in_=gb_e[:, :])
            nc.sync.dma_start(out=sel8_sb, in_=sel8_e[:, :])
            nc.sync.dma_start(out=bo_sb, in_=bo_e[:, :])
            for p in range(1, 3):
                for t in range(4):
                    ebt_load(p, t)
            nc.sync.dma_start(out=wgt_sb, in_=wgt_e[:, :, :])
            for t in range(4):
                ebt_load(3, t)
            nc.sync.dma_start(out=wot_sb, in_=wot_e[:, :, :])
            nc.sync.dma_start(out=wot2_sb, in_=wot2_e[:, :])

            # ones column of v_aug (col D of each head's 65-wide group)
            nc.vector.memset(
                vaug_sb.rearrange("p c (h e) -> p c h e", h=H)[:, :, :, D : D + 1],
                1.0,
            )

            # ---------------- projection helpers ----------------
            DR = mybir.MatmulPerfMode.DoubleRow
            WS = 1.0 / 32.0  # host weight scale compensation

            def kt_proj(mo, no):
                ps = ps_pr.tile([P, 512], F32, tag="pr", name=f"ps_k{mo}{no}")
                for ko in range(4):
                    nc.tensor.matmul(
                        ps,
                        lhsT=wkt_sb[:, ko, mo * P : (mo + 1) * P],
                        rhs=xt_sb[:, ko, no * 512 : (no + 1) * 512],
                        start=(ko == 0),
                        stop=(ko == 3),
                    )
                nc.vector.tensor_copy(
                    out=kt_sb[:, mo, no * 512 : (no + 1) * 512], in_=ps
                )

            def qt_proj(mo, act_evac=False):
                ps = ps_pr.tile([P, 512], F32, tag="pr", name=f"ps_q{mo}")
                for ko in range(4):
                    nc.tensor.matmul(
                        ps,
                        lhsT=wqt_sb[:, ko, mo * P : (mo + 1) * P],
                        rhs=xt_sb[:, ko, 0:NQ],
                        start=(ko == 0),
                        stop=(ko == 3),
                    )
                if act_evac:
                    nc.scalar.activation(
                        out=qt_sb[:, mo, :],
                        in_=ps,
                        func=AF.Identity,
                        bias=bqs_sb[:, mo : mo + 1],
                    )
                else:
                    nc.vector.tensor_scalar_add(
                        out=qt_sb[:, mo, :], in0=ps, scalar1=bqs_sb[:, mo : mo + 1]
                    )

            def v_proj(c):
                ps = ps_pr.tile([P, 512], F32, tag="pr", name=f"ps_v{c}")
                for ko in range(4):
                    nc.tensor.matmul(
                        ps,
                        lhsT=xt_sb[:, ko, c * P : (c + 1) * P],
                        rhs=wvt_sb[:, ko, :],
                        start=(ko == 0),
                        stop=(ko == 3),
                    )
                dst = vaug_sb.rearrange("p c (h e) -> p c h e", h=H)[:, c, :, 0:D]
                nc.vector.tensor_copy(
                    out=dst, in_=ps.rearrange("p (h d) -> p h d", h=H)
                )

            # ---------------- attention building blocks ----------------
            def logits_step(p, c):
                """One key-chunk step: the pair's 2 K=64 logit MMs written
                into ONE shared s-tile ([:,h01,:]) so they become adjacent
                and slot-gated by the same event -> the PE row-group packing
                engages (both run concurrently, ~2x). One exp for both."""
                s = ps_s.tile([P, 2, 512], F32, tag="s", name=f"s_{p}_{c}")
                for h01 in range(2):
                    d0 = h01 * D
                    nc.tensor.matmul(
                        s[:, h01, :],
                        lhsT=kt_sb[d0 : d0 + D, p, c * P : (c + 1) * P],
                        rhs=qt_sb[d0 : d0 + D, p, :],
                        start=True,
                        stop=True,
                    )
                e = etmp.tile([P, 2, 512], BF16, tag="e", name=f"e_{p}_{c}", bufs=6)
                nc.scalar.activation(out=e, in_=s, func=AF.Exp)
                return e

            def mult_step(p, c, e):
                r0 = p * 16 + c * 2
                nc.vector.tensor_tensor(
                    e, e, ebt_sb[:, r0 : r0 + 2, :], OP.mult
                )

            def pv_step(p, c, e, pv_ps):
                for h01 in range(2):
                    h = 2 * p + h01
                    nc.tensor.matmul(
                        pv_ps[h],
                        lhsT=vaug_sb[:, c, h * (D + 1) : (h + 1) * (D + 1)],
                        rhs=e[:, h01, :],
                        start=(c == 0),
                        stop=(c == 7),
                    )

            def pv_alloc(p):
                return {
                    h: ps_pv.tile([D + 1, NQ], F32, tag="pv", name=f"pv_{h}")
                    for h in (2 * p, 2 * p + 1)
                }

            def oraw_evac(pv_ps):
                # ACT copies, emitted at pair boundaries (deps already met)
                for h, ps in pv_ps.items():
                    nc.scalar.copy(out=oraw_sb[:, h, :], in_=ps)

            # ---------------- softmax normalization ----------------
            def norm_gather(h0, cnt):
                den_bf = singles.tile([cnt, NQ], BF16, name=f"denb_{h0}")
                nc.sync.dma_start(
                    out=den_bf, in_=oraw_sb[D : D + 1, h0 : h0 + cnt, :]
                )
                return den_bf

            def norm_recip(h0, cnt, den_bf):
                den_f = singles.tile([cnt, NQ], F32, name=f"denf_{h0}")
                recipf = singles.tile([cnt, NQ], F32, name=f"recipf_{h0}")
                recipb = singles.tile([cnt, NQ], BF16, name=f"recipb_{h0}")
                nc.vector.tensor_copy(out=den_f, in_=den_bf)
                nc.vector.reciprocal_approx_fast(out=recipf, in_=den_f)
                # bf16 for the rbc matmul: an fp32 matmul runs two LOW/HIGH
                # passes (~1.1us + double LDWEIGHTS) and clogs the PE queue
                nc.vector.tensor_copy(out=recipb, in_=recipf)
                return recipb

            def norm_head(h0, cnt, i, recipb):
                """Broadcast 1/den for head h0+i over 64 partitions (PE) and
                scale oraw (DVE, straight from PSUM). Even heads land in
                ofin partitions 0-63 directly (same lanes); odd heads stage
                in osc and get relocated to partitions 64-127 by DMA."""
                h = h0 + i
                rbc = ps_pr.tile([D, NQ], F32, tag="pr", name=f"rbc_{h}")
                nc.tensor.matmul(
                    rbc,
                    lhsT=sel8_sb[0:cnt, i * D : (i + 1) * D],
                    rhs=recipb,
                    start=True,
                    stop=True,
                )
                dst = ofin_sb[0:D, h // 2, :] if h % 2 == 0 else osc_sb[:, h, :]
                nc.vector.tensor_tensor(dst, oraw_sb[0:D, h, :], rbc, OP.mult)

            def norm_reloc(h0, cnt):
                # odd heads -> partitions 64-127 (evens were written direct)
                mo0 = h0 // 2
                nmo = cnt // 2
                odds = osc_sb[:, h0 : h0 + cnt, :].rearrange(
                    "p (m t) q -> p m t q", t=2
                )
                nc.sync.dma_start(
                    out=ofin_sb[D:P, mo0 : mo0 + nmo, :], in_=odds[:, :, 1, :]
                )

            # ---------------- gate + to_out ----------------
            def gate_proj(mo):
                ps = ps_pr.tile([P, 512], F32, tag="pr", name=f"ps_g{mo}")
                for kop in (0, 2):
                    nc.tensor.matmul(
                        ps,
                        lhsT=wgt_sb[:, kop : kop + 2, mo * P : (mo + 1) * P],
                        rhs=xt8_sb[:, kop : kop + 2, :],
                        start=(kop == 0),
                        stop=(kop == 2),
                        perf_mode=DR,
                    )
                # gate = 0.5*tanh(0.5*z + 0.5*gb) + 0.5 == sigmoid(z + gb)
                # (tanh shares the exp table set; sigmoid would thrash it)
                nc.scalar.activation(
                    out=gate_sb[:, mo, :],
                    in_=ps,
                    func=AF.Tanh,
                    bias=gb_sb[:, mo : mo + 1],
                    scale=0.5 * WS,
                )
                nc.vector.tensor_scalar(
                    out=gate_sb[:, mo, :],
                    in0=gate_sb[:, mo, :],
                    scalar1=0.5,
                    scalar2=0.5,
                    op0=OP.mult,
                    op1=OP.add,
                )

            # to_out: 4 per-mo accumulators packed as halves of 2 shared
            # 2-bank PSUM tiles, so ko0-2 partials for ALL mo run before the
            # tail's norm-B2 chain; only the 4 ko3 matmuls + stt remain
            # serialized behind the last normalization.
            to_ps = {}

            def toout_slot(mo):
                pairi = mo // 2
                if pairi not in to_ps:
                    to_ps[pairi] = ps_s.tile(
                        [P, 2, 512], F32, tag="s", name=f"to_{pairi}"
                    )
                return to_ps[pairi][:, mo % 2, :]

            def toout_partial(mo, kos):
                ps = toout_slot(mo)
                for ko in kos:
                    nc.tensor.matmul(
                        ps,
                        lhsT=wot_sb[:, ko, mo * P : (mo + 1) * P],
                        rhs=ofin_sb[:, ko, :],
                        start=(ko == 0),
                        stop=(ko == 3),
                    )

            def toout_finish(mo):
                # out = (ps + bo) * gate, single fused DVE op, then DMA out
                nc.vector.scalar_tensor_tensor(
                    out=outf_sb[:, mo, :],
                    in0=toout_slot(mo),
                    scalar=bo_sb[:, mo : mo + 1],
                    in1=gate_sb[:, mo, :],
                    op0=OP.add,
                    op1=OP.mult,
                )
                nc.sync.dma_start(out=out_e[:, mo, :], in_=outf_sb[:, mo, :])

            # ================= schedule =================
            # Software pipeline over global steps g (8 key-chunk steps per
            # pair): logits+exp lead; mult+PV trail by LAG=3 steps so the
            # exp->mult->PV chain never head-of-line blocks the PE while ACT
            # stays 100% fed. Projections/norm/gate fillers at fixed steps.
            kt_proj(0, 0)
            qt_proj(0, act_evac=True)
            kt_proj(0, 1)

            LAG = 3
            e_tiles = {}  # g -> e tile
            pv = {}  # p -> {h: psum}
            state = {}

            def lead(g):
                p, c = divmod(g, 8)
                if p > 3:
                    return
                e_tiles[g] = logits_step(p, c)

            def trail(g):
                p, c = divmod(g - LAG, 8)
                if c == 0:
                    # evacuate the previous pair BEFORE recycling its pv slots
                    if p > 0:
                        oraw_evac(pv[p - 1])
                    pv[p] = pv_alloc(p)
                mult_step(p, c, e_tiles[g - LAG])
                pv_step(p, c, e_tiles[g - LAG], pv[p])

            fillers = {
                # v fillers start at g=2 so their wvt DMA wait never sits at
                # the head of the PE queue in front of the first logit steps
                2: lambda: v_proj(0),
                3: lambda: v_proj(1),
                4: lambda: (v_proj(2), kt_proj(1, 0)),
                5: lambda: (v_proj(3), v_proj(4), qt_proj(1)),
                6: lambda: (v_proj(5), v_proj(6)),
                7: lambda: v_proj(7),
                9: lambda: kt_proj(1, 1),
                11: lambda: kt_proj(2, 0),
                13: lambda: kt_proj(2, 1),
                14: lambda: qt_proj(2),
                19: lambda: kt_proj(3, 0),
                21: lambda: kt_proj(3, 1),
                22: lambda: qt_proj(3),
                # norm A (heads 0-3): oraw p0 lands in trail g=11, oraw p1 in
                # trail g=19 -> gather A at g=20; rbc heads a step after the
                # recip chain so the PE never waits on the DVE.
                20: lambda: state.update(denA=norm_gather(0, 4)),
                23: lambda: state.update(recipA=norm_recip(0, 4, state["denA"])),
                24: lambda: norm_head(0, 4, 0, state["recipA"]),
                25: lambda: norm_head(0, 4, 1, state["recipA"]),
                26: lambda: (norm_head(0, 4, 2, state["recipA"]), gate_proj(0)),
                27: lambda: (
                    norm_head(0, 4, 3, state["recipA"]),
                    norm_reloc(0, 4),
                ),
                # oraw p2 lands at trail g=27; B1 = heads 4-5
                28: lambda: (state.update(denB1=norm_gather(4, 2)), gate_proj(1)),
                29: lambda: state.update(recipB1=norm_recip(4, 2, state["denB1"])),
                30: lambda: gate_proj(2),
                31: lambda: (
                    norm_head(4, 2, 0, state["recipB1"]),
                    norm_head(4, 2, 1, state["recipB1"]),
                    norm_reloc(4, 2),
                ),
                # lead stream is done after g=31; to_out partials ko0-2 fill
                # the trail-only steps (ps_s slots now free of s-tiles)
                32: lambda: (toout_partial(0, (0, 1, 2)), toout_partial(1, (0, 1, 2))),
                33: lambda: (toout_partial(2, (0, 1, 2)), toout_partial(3, (0, 1, 2))),
            }

            for g in range(8 * 4 + LAG):
                lead(g)
                if g >= LAG:
                    trail(g)
                f = fillers.get(g)
                if f is not None:
                    f()

            # ---- tail ----
            # oraw p3 + den gather emitted BEFORE gate3's tanh so the ACT
            # queue doesn't delay the norm-B2 chain; gate3 follows.
            oraw_evac(pv[3])
            denB2 = norm_gather(6, 2)
            gate_proj(3)
            recipB2 = norm_recip(6, 2, denB2)
            norm_head(6, 2, 0, recipB2)  # h6 -> ofin[0:64, 3] direct
            norm_head(6, 2, 1, recipB2)  # h7 -> osc[:, 7]
            # no reloc for h7: to_out's ko3 splits into two K=64 matmuls --
            # the odd half reads osc directly against host-relocated wot2
            # (cuts ~2us of SBUF-relocation DMA latency off the tail)
            for mo in range(4):
                ms = slice(mo * P, (mo + 1) * P)
                nc.tensor.matmul(
                    toout_slot(mo),
                    lhsT=wot_sb[0:D, 3, ms],
                    rhs=ofin_sb[0:D, 3, :],
                    start=False,
                    stop=False,
                )
                nc.tensor.matmul(
                    toout_slot(mo),
                    lhsT=wot2_sb[:, ms],
                    rhs=osc_sb[:, 7, :],
                    start=False,
                    stop=True,
                )
                toout_finish(mo)

    nc.compile()
    return nc


def make_in_maps(q_x, attn_bias, Wq, bq, Wk, Wv, Wo, bo, Wg, bg, gating_bias):
    import ml_dtypes

    bf16 = ml_dtypes.bfloat16
    fp8 = ml_dtypes.float8_e4m3
    scale = np.float32(D) ** -0.5

    def swz(a2d):
        """[512, M] -> [128, 4, M] SBUF layout (partition-inner on dim 0)."""
        m = a2d.shape[1]
        return np.ascontiguousarray(a2d.reshape(4, P, m).transpose(1, 0, 2))

    # gate weights in fp8 scaled x32 (the tanh evac divides it back out);
    # q/k stay bf16 -- fp8 there costs ~1.4% output error (softmax-weight
    # noise does not average down)
    wqt = swz(Wq.T.astype(np.float32) * scale).astype(bf16)
    wkt = swz(np.asarray(Wk.T, dtype=np.float32)).astype(bf16)
    wgt = (swz(np.asarray(Wg.T, np.float32)) * 32.0).astype(fp8)
    wvt = swz(np.asarray(Wv.T, dtype=np.float32)).astype(bf16)
    wot = swz(np.asarray(Wo.T, dtype=np.float32)).astype(bf16)
    # h7's Wo rows staged at partitions 0-63 (reloc-free ko3-odd matmul)
    wot2 = np.ascontiguousarray(np.asarray(Wo.T, np.float32)[7 * D : 8 * D, :]).astype(
        bf16
    )
    bqs = np.ascontiguousarray((bq * scale).reshape(4, P).T).astype(np.float32)
    bo_ = np.ascontiguousarray(np.asarray(bo).reshape(4, P).T).astype(np.float32)
    gb = np.ascontiguousarray(
        (0.5 * (bg + gating_bias)).reshape(4, P).T
    ).astype(np.float32)
    sel8 = np.repeat(np.eye(H, dtype=np.float32), D, axis=1).astype(bf16)

    in_maps = []
    for c in range(8):
        b, half = c // 2, c % 2
        o0, o1 = half * NQ, (1 - half) * NQ
        x = np.asarray(q_x[b], dtype=np.float32)  # [N, CQ]
        # keys permuted: own query-row half first (q reads prefix of xt)
        xp = np.concatenate([x[o0 : o0 + NQ], x[o1 : o1 + NQ]], axis=0)
        xts = swz(np.ascontiguousarray(xp.T))
        xt = xts.astype(bf16)  # [128, 4, N]
        xt8 = np.ascontiguousarray(xts[:, :, 0:NQ]).astype(fp8)  # own rows
        # ebt[p, pair*16 + c*2 + h01, q] = exp(bias)[2*pair+h01,
        #   perm_key[c*128+p], own_row q]
        eb = np.exp(np.asarray(attn_bias[b, :, o0 : o0 + NQ, :], np.float32))
        ebp = np.concatenate(
            [eb[:, :, o0 : o0 + NQ], eb[:, :, o1 : o1 + NQ]], axis=2
        )  # [H, q, k(perm)]
        ebt = (
            ebp.transpose(0, 2, 1)  # [H, k, q]
            .reshape(4, 2, 8, P, NQ)  # [pair, h01, c, p, q]
            .transpose(3, 0, 2, 1, 4)  # [p, pair, c, h01, q]
            .reshape(P, H * 8, NQ)
        )
        ebt = np.ascontiguousarray(ebt).astype(bf16)
        in_maps.append(
            {
                "xt": xt,
                "xt8": xt8,
                "ebt": ebt,
                "wqt": wqt,
                "wkt": wkt,
                "wvt": wvt,
                "wot": wot,
                "wot2": wot2,
                "wgt": wgt,
                "bqs": bqs,
                "bo": bo_,
                "gb": gb,
                "sel8": sel8,
            }
        )
    return in_maps


_NC_CACHE = None


def kernel(**inputs) -> np.ndarray:
    global _NC_CACHE
    from concourse.bass_utils import run_bass_kernel_spmd

    if _NC_CACHE is None:
        _NC_CACHE = build_nc()
    nc = _NC_CACHE
    in_maps = make_in_maps(**inputs)
    trace = bool(int(os.environ.get("BASS_KERNEL_TRACE", "0")))
    last_exc = None
    for attempt in range(3):
        try:
            res = run_bass_kernel_spmd(nc, in_maps, list(range(8)), trace=trace)
            break
        except Exception as exc:  # transient NRT/axon device hiccups
            last_exc = exc
            import time

            time.sleep(10 * (attempt + 1))
    else:
        raise last_exc
    kernel.last_result = res
    out = np.empty((B, N, CQ), dtype=np.float32)
    for c in range(8):
        b, half = c // 2, c % 2
        # res "out" is [128, 4, NQ]: out^T[cq=o*128+i, q] at [i, o, q]
        o = res.results[c]["out"]
        out[b, half * NQ : (half + 1) * NQ, :] = (
            o.transpose(1, 0, 2).reshape(CQ, NQ).T.astype(np.float32)
        )
    return out


# revision 53
# speedup vs baseline: 1.0149x; 1.0149x over previous
"""Trainium2 Bass kernel for nn_Attention_79164837199973 (v8).

Bias-augmented multi-head self-attention with sigmoid gating.
B=4, N=1024, CQ=CH=512, H=8, D=64.  ~79us HW (vs 104us baseline).

Sharding (8 cores, no collectives): core c -> batch b=c//2, query-row half
r=c%2 (512 rows). Keys are PERMUTED per core (own query-row half first) so
the q-projection reads a prefix slice of the same x^T tile -- no separate
xtq DMA. kt/ebt/vaug all use the permuted key order (softmax sum order is
irrelevant).

Structure (trace-driven, v7/v8):
  - software pipeline over 8 single-key-chunk steps per head-pair: the
    logits+exp stream leads, mult+PV trails by LAG=3 steps, with
    projection/norm/gate/to_out matmuls placed as fillers at fixed steps.
    The PE never idles >3.4us, so the HAM clock-gate stays at 2.4 GHz
    (the v6 schedule spent ~38us re-throttled at 1.2 GHz).
  - each step's two K=64 logit matmuls write one SHARED s-tile, making
    them adjacent and slot-gated by the same event -> the PE row-group
    packing engages (both run concurrently; ~2x on the logits block).
  - engine balance: ACT = exps + qt0 evac + oraw evacs + tanh-gate (tanh
    shares the exp ACT-table set; Sigmoid would thrash the table -- gate
    = 0.5*tanh(0.5 z + 0.5 gb)+0.5). DVE = kt/v/qt1-3 evacs + bias-mults
    + norm chain + fused (ps+bo)*gate to_out evacuation.
  - gate projection in fp8e4 DoubleRow (2 matmuls, weights x32 host-scaled
    against denormals, descaled in the tanh). q/k/v stay bf16: fp8 there
    costs 1.4%+ output error (softmax-weight noise does not average down).
  - norm: reciprocal_approx_fast on f32, bf16 recip-broadcast matmul (an
    fp32 matmul runs two LOW/HIGH passes ~1.1us), product taken straight
    from PSUM. 3 batches (h0-3 mid, h4-5 late-mid, h6-7 tail). Even heads
    write ofin directly (same lanes); odd heads relocate by DMA -- except
    h7, whose to_out ko3 contribution is a separate K=64 matmul against
    host-relocated wot2 rows (kills the tail's 2us relocation DMA).
  - to_out: per-mo accumulators packed as halves of two 2-bank PSUM tiles;
    ko0-2 run as trail-only-step fillers, only ko3 + the fused stt remain
    behind the last normalization. Output dtype bf16 (host upcasts).
  - PE pre-warm matmuls must be K=128: the HAM watches ARRAY activity, a
    K=1 matmul engages 1 of 128 rows and never un-throttles the clock.
"""

import os
import sys

sys.path.insert(0, "/opt/trn_rl_repo")

import numpy as np

import concourse.bass as bass
import concourse.tile as tile
from concourse import bacc, mybir

B, N, CQ, CH, H = 4, 1024, 512, 512, 8
D = CH // H  # 64
NQ = N // 2  # 512 query rows per core
P = 128
F32 = mybir.dt.float32
BF16 = mybir.dt.bfloat16
AF = mybir.ActivationFunctionType
OP = mybir.AluOpType


def build_nc():
    nc = bacc.Bacc("TRN2", target_bir_lowering=False, debug=False, num_devices=8)

    # ---- DRAM parameters, already in SBUF layout (host pre-swizzled) ----
    # kt/qt/gate projections run fp8e4 DoubleRow (weights host-scaled by 32
    # against denormals, descaled in the evacuations); v stays bf16.
    FP8 = mybir.dt.float8e4
    xt_e = nc.declare_dram_parameter("xt", [P, 4, N], BF16, isOutput=False)
    xt8_e = nc.declare_dram_parameter("xt8", [P, 4, NQ], FP8, isOutput=False)
    ebt_e = nc.declare_dram_parameter("ebt", [P, H * 8, NQ], BF16, isOutput=False)
    wqt_e = nc.declare_dram_parameter("wqt", [P, 4, CH], BF16, isOutput=False)
    wkt_e = nc.declare_dram_parameter("wkt", [P, 4, CH], BF16, isOutput=False)
    wvt_e = nc.declare_dram_parameter("wvt", [P, 4, CH], BF16, isOutput=False)
    wot_e = nc.declare_dram_parameter("wot", [P, 4, CQ], BF16, isOutput=False)
    wot2_e = nc.declare_dram_parameter("wot2", [D, CQ], BF16, isOutput=False)
    wgt_e = nc.declare_dram_parameter("wgt", [P, 4, CQ], FP8, isOutput=False)
    bqs_e = nc.declare_dram_parameter("bqs", [P, 4], F32, isOutput=False)
    bo_e = nc.declare_dram_parameter("bo", [P, 4], F32, isOutput=False)
    gb_e = nc.declare_dram_parameter("gb", [P, 4], F32, isOutput=False)
    sel8_e = nc.declare_dram_parameter("sel8", [H, H * D], BF16, isOutput=False)
    out_e = nc.declare_dram_parameter("out", [P, 4, NQ], BF16, isOutput=True)

    with tile.TileContext(nc) as tc:
        with (
            tc.tile_pool(name="singles", bufs=1) as singles,
            tc.tile_pool(name="etmp", bufs=3) as etmp,
            tc.tile_pool(name="ps_s", bufs=2, space="PSUM") as ps_s,
            tc.tile_pool(name="ps_pv", bufs=2, space="PSUM") as ps_pv,
            tc.tile_pool(name="ps_pr", bufs=2, space="PSUM") as ps_pr,
        ):
            # ---- persistent SBUF tiles ----
            xt_sb = singles.tile([P, 4, N], BF16)
            xt8_sb = singles.tile([P, 4, NQ], FP8)
            wqt_sb = singles.tile([P, 4, CH], BF16)
            wkt_sb = singles.tile([P, 4, CH], BF16)
            wvt_sb = singles.tile([P, 4, CH], BF16)
            wot_sb = singles.tile([P, 4, CQ], BF16)
            wot2_sb = singles.tile([D, CQ], BF16)
            wgt_sb = singles.tile([P, 4, CQ], FP8)
            bqs_sb = singles.tile([P, 4], F32)
            bo_sb = singles.tile([P, 4], F32)
            gb_sb = singles.tile([P, 4], F32)
            sel8_sb = singles.tile([H, H * D], BF16)
            ebt_sb = singles.tile([P, H * 8, NQ], BF16)
            kt_sb = singles.tile([P, 4, N], BF16)
            qt_sb = singles.tile([P, 4, NQ], BF16)
            vaug_sb = singles.tile([P, 8, H * (D + 1)], BF16)
            oraw_sb = singles.tile([D + 1, H, NQ], BF16)  # 0-63 o, row 64 den
            osc_sb = singles.tile([D, H, NQ], BF16)  # normalized o
            ofin_sb = singles.tile([P, 4, NQ], BF16)  # head-merged o^T
            gate_sb = singles.tile([P, 4, NQ], BF16)
            outf_sb = singles.tile([P, 4, NQ], BF16)
            warm_sb = singles.tile([1, 8], F32)
            warmo_sb = singles.tile([1, 8], BF16)

            # force the exp table load off the critical path (first ACT op)
            nc.vector.memset(warm_sb, 0.0)
            nc.scalar.activation(out=warmo_sb, in_=warm_sb, func=AF.Exp)

            # PE pre-warm during the input-DMA wait: K must be 128 (the HAM
            # watches ARRAY activity -- a K=1 matmul engages 1 of 128 rows
            # and never un-throttles the clock), and N small so the stream
            # does not delay the first real matmuls.
            warm_big = singles.tile([P, 8], BF16)
            warmr_sb = singles.tile([P, P], BF16)
            nc.vector.memset(warm_big, 1.0)
            nc.vector.memset(warmr_sb, 1.0)
            warm_ps = ps_pr.tile([8, P], F32, tag="pr", name="warm_ps")
            for _ in range(24):
                nc.tensor.matmul(
                    warm_ps, lhsT=warm_big, rhs=warmr_sb, start=True, stop=True
                )

            # ---- input DMAs: FIFO order = priority order; mo0 slices of
            # wkt/wqt split out so kt(0,0)/qt(0) start ~4us earlier ----
            nc.sync.dma_start(out=wkt_sb[:, :, 0:P], in_=wkt_e[:, :, 0:P])
            nc.sync.dma_start(out=xt_sb[:, :, 0:NQ], in_=xt_e[:, :, 0:NQ])
            nc.sync.dma_start(out=wqt_sb[:, :, 0:P], in_=wqt_e[:, :, 0:P])
            nc.sync.dma_start(out=bqs_sb, in_=bqs_e[:, :])
            nc.sync.dma_start(out=wkt_sb[:, :, P:CH], in_=wkt_e[:, :, P:CH])
            nc.sync.dma_start(out=xt_sb[:, :, NQ:N], in_=xt_e[:, :, NQ:N])
            nc.sync.dma_start(out=wvt_sb, in_=wvt_e[:, :, :])
            nc.sync.dma_start(out=wqt_sb[:, :, P:CH], in_=wqt_e[:, :, P:CH])
            nc.sync.dma_start(out=xt8_sb, in_=xt8_e[:, :, :])

            def ebt_load(p, t):
                r0 = p * 16 + t * 4
                nc.sync.dma_start(
                    out=ebt_sb[:, r0 : r0 + 4, :], in_=ebt_e[:, r0 : r0 + 4, :]
                )

            for t in range(4):
                ebt_load(0, t)
            nc.sync.dma_start(out=gb_sb, in_=gb_e[:, :])
            nc.sync.dma_start(out=sel8_sb, in_=sel8_e[:, :])
            nc.sync.dma_start(out=bo_sb, in_=bo_e[:, :])
            for p in range(1, 3):
                for t in range(4):
                    ebt_load(p, t)
            nc.sync.dma_start(out=wgt_sb, in_=wgt_e[:, :, :])
            for t in range(4):
                ebt_load(3, t)
            nc.sync.dma_start(out=wot_sb, in_=wot_e[:, :, :])
            nc.sync.dma_start(out=wot2_sb, in_=wot2_e[:, :])

            # ones column of v_aug (col D of each head's 65-wide group)
            nc.vector.memset(
                vaug_sb.rearrange("p c (h e) -> p c h e", h=H)[:, :, :, D : D + 1],
                1.0,
            )

            # ---------------- projection helpers ----------------
            DR = mybir.MatmulPerfMode.DoubleRow
            WS = 1.0 / 32.0  # host weight scale compensation

            def kt_proj(mo, no):
                ps = ps_pr.tile([P, 512], F32, tag="pr", name=f"ps_k{mo}{no}")
                for ko in range(4):
                    nc.tensor.matmul(
                        ps,
                        lhsT=wkt_sb[:, ko, mo * P : (mo + 1) * P],
                        rhs=xt_sb[:, ko, no * 512 : (no + 1) * 512],
                        start=(ko == 0),
                        stop=(ko == 3),
                    )
                nc.vector.tensor_copy(
                    out=kt_sb[:, mo, no * 512 : (no + 1) * 512], in_=ps
                )

            def qt_proj(mo, act_evac=False):
                ps = ps_pr.tile([P, 512], F32, tag="pr", name=f"ps_q{mo}")
                for ko in range(4):
                    nc.tensor.matmul(
                        ps,
                        lhsT=wqt_sb[:, ko, mo * P : (mo + 1) * P],
                        rhs=xt_sb[:, ko, 0:NQ],
                        start=(ko == 0),
                        stop=(ko == 3),
                    )
                if act_evac:
                    nc.scalar.activation(
                        out=qt_sb[:, mo, :],
                        in_=ps,
                        func=AF.Identity,
                        bias=bqs_sb[:, mo : mo + 1],
                    )
                else:
                    nc.vector.tensor_scalar_add(
                        out=qt_sb[:, mo, :], in0=ps, scalar1=bqs_sb[:, mo : mo + 1]
                    )

            def v_proj(c):
                ps = ps_pr.tile([P, 512], F32, tag="pr", name=f"ps_v{c}")
                for ko in range(4):
                    nc.tensor.matmul(
                        ps,
                        lhsT=xt_sb[:, ko, c * P : (c + 1) * P],
                        rhs=wvt_sb[:, ko, :],
                        start=(ko == 0),
                        stop=(ko == 3),
                    )
                dst = vaug_sb.rearrange("p c (h e) -> p c h e", h=H)[:, c, :, 0:D]
                nc.vector.tensor_copy(
                    out=dst, in_=ps.rearrange("p (h d) -> p h d", h=H)
                )

            # ---------------- attention building blocks ----------------
            def logits_step(p, c):
                """One key-chunk step: the pair's 2 K=64 logit MMs written
                into ONE shared s-tile ([:,h01,:]) so they become adjacent
                and slot-gated by the same event -> the PE row-group packing
                engages (both run concurrently, ~2x). One exp for both."""
                s = ps_s.tile([P, 2, 512], F32, tag="s", name=f"s_{p}_{c}")
                for h01 in range(2):
                    d0 = h01 * D
                    nc.tensor.matmul(
                        s[:, h01, :],
                        lhsT=kt_sb[d0 : d0 + D, p, c * P : (c + 1) * P],
                        rhs=qt_sb[d0 : d0 + D, p, :],
                        start=True,
                        stop=True,
                    )
                e = etmp.tile([P, 2, 512], BF16, tag="e", name=f"e_{p}_{c}", bufs=6)
                nc.scalar.activation(out=e, in_=s, func=AF.Exp)
                return e

            def mult_step(p, c, e):
                r0 = p * 16 + c * 2
                nc.vector.tensor_tensor(
                    e, e, ebt_sb[:, r0 : r0 + 2, :], OP.mult
                )

            def pv_step(p, c, e, pv_ps):
                for h01 in range(2):
                    h = 2 * p + h01
                    nc.tensor.matmul(
                        pv_ps[h],
                        lhsT=vaug_sb[:, c, h * (D + 1) : (h + 1) * (D + 1)],
                        rhs=e[:, h01, :],
                        start=(c == 0),
                        stop=(c == 7),
                    )

            def pv_alloc(p):
                return {
                    h: ps_pv.tile([D + 1, NQ], F32, tag="pv", name=f"pv_{h}")
                    for h in (2 * p, 2 * p + 1)
                }

            def oraw_evac(pv_ps):
                # ACT copies, emitted at pair boundaries (deps already met)
                for h, ps in pv_ps.items():
                    nc.scalar.copy(out=oraw_sb[:, h, :], in_=ps)

            # ---------------- softmax normalization ----------------
            def norm_gather(h0, cnt):
                den_bf = singles.tile([cnt, NQ], BF16, name=f"denb_{h0}")
                nc.sync.dma_start(
                    out=den_bf, in_=oraw_sb[D : D + 1, h0 : h0 + cnt, :]
                )
                return den_bf

            def norm_recip(h0, cnt, den_bf):
                den_f = singles.tile([cnt, NQ], F32, name=f"denf_{h0}")
                recipf = singles.tile([cnt, NQ], F32, name=f"recipf_{h0}")
                recipb = singles.tile([cnt, NQ], BF16, name=f"recipb_{h0}")
                nc.vector.tensor_copy(out=den_f, in_=den_bf)
                nc.vector.reciprocal_approx_fast(out=recipf, in_=den_f)
                # bf16 for the rbc matmul: an fp32 matmul runs two LOW/HIGH
                # passes (~1.1us + double LDWEIGHTS) and clogs the PE queue
                nc.vector.tensor_copy(out=recipb, in_=recipf)
                return recipb

            def norm_head(h0, cnt, i, recipb):
                """Broadcast 1/den for head h0+i over 64 partitions (PE) and
                scale oraw (DVE, straight from PSUM). Even heads land in
                ofin partitions 0-63 directly (same lanes); odd heads stage
                in osc and get relocated to partitions 64-127 by DMA."""
                h = h0 + i
                rbc = ps_pr.tile([D, NQ], F32, tag="pr", name=f"rbc_{h}")
                nc.tensor.matmul(
                    rbc,
                    lhsT=sel8_sb[0:cnt, i * D : (i + 1) * D],
                    rhs=recipb,
                    start=True,
                    stop=True,
                )
                dst = ofin_sb[0:D, h // 2, :] if h % 2 == 0 else osc_sb[:, h, :]
                nc.vector.tensor_tensor(dst, oraw_sb[0:D, h, :], rbc, OP.mult)

            def norm_reloc(h0, cnt):
                # odd heads -> partitions 64-127 (evens were written direct)
                mo0 = h0 // 2
                nmo = cnt // 2
                odds = osc_sb[:, h0 : h0 + cnt, :].rearrange(
                    "p (m t) q -> p m t q", t=2
                )
                nc.sync.dma_start(
                    out=ofin_sb[D:P, mo0 : mo0 + nmo, :], in_=odds[:, :, 1, :]
                )

            # ---------------- gate + to_out ----------------
            def gate_proj(mo):
                ps = ps_pr.tile([P, 512], F32, tag="pr", name=f"ps_g{mo}")
                for kop in (0, 2):
                    nc.tensor.matmul(
                        ps,
                        lhsT=wgt_sb[:, kop : kop + 2, mo * P : (mo + 1) * P],
                        rhs=xt8_sb[:, kop : kop + 2, :],
                        start=(kop == 0),
                        stop=(kop == 2),
                        perf_mode=DR,
                    )
                # gate = 0.5*tanh(0.5*z + 0.5*gb) + 0.5 == sigmoid(z + gb)
                # (tanh shares the exp table set; sigmoid would thrash it)
                nc.scalar.activation(
                    out=gate_sb[:, mo, :],
                    in_=ps,
                    func=AF.Tanh,
                    bias=gb_sb[:, mo : mo + 1],
                    scale=0.5 * WS,
                )
                nc.vector.tensor_scalar(
                    out=gate_sb[:, mo, :],
                    in0=gate_sb[:, mo, :],
                    scalar1=0.5,
                    scalar2=0.5,
                    op0=OP.mult,
                    op1=OP.add,
                )

            # to_out: 4 per-mo accumulators packed as halves of 2 shared
            # 2-bank PSUM tiles, so ko0-2 partials for ALL mo run before the
            # tail's norm-B2 chain; only the 4 ko3 matmuls + stt remain
            # serialized behind the last normalization.
            to_ps = {}

            def toout_slot(mo):
                pairi = mo // 2
                if pairi not in to_ps:
                    to_ps[pairi] = ps_s.tile(
                        [P, 2, 512], F32, tag="s", name=f"to_{pairi}"
                    )
                return to_ps[pairi][:, mo % 2, :]

            def toout_partial(mo, kos):
                ps = toout_slot(mo)
                for ko in kos:
                    nc.tensor.matmul(
                        ps,
                        lhsT=wot_sb[:, ko, mo * P : (mo + 1) * P],
                        rhs=ofin_sb[:, ko, :],
                        start=(ko == 0),
                        stop=(ko == 3),
                    )

            def toout_finish(mo):
                # out = (ps + bo) * gate, single fused DVE op, then DMA out
                nc.vector.scalar_tensor_tensor(
                    out=outf_sb[:, mo, :],
                    in0=toout_slot(mo),
                    scalar=bo_sb[:, mo : mo + 1],
                    in1=gate_sb[:, mo, :],
                    op0=OP.add,
                    op1=OP.mult,
                )
                nc.sync.dma_start(out=out_e[:, mo, :], in_=outf_sb[:, mo, :])

            # ================= schedule =================
            # Software pipeline over global steps g (8 key-chunk steps per
            # pair): logits+exp lead; mult+PV trail by LAG=3 steps so the
            # exp->mult->PV chain never head-of-line blocks the PE while ACT
            # stays 100% fed. Projections/norm/gate fillers at fixed steps.
            kt_proj(0, 0)
            qt_proj(0, act_evac=True)
            kt_proj(0, 1)

            LAG = 3
            e_tiles = {}  # g -> e tile
            pv = {}  # p -> {h: psum}
            state = {}

            def lead(g):
                p, c = divmod(g, 8)
                if p > 3:
                    return
                e_tiles[g] = logits_step(p, c)

            def trail(g):
                p, c = divmod(g - LAG, 8)
                if c == 0:
                    # evacuate the previous pair BEFORE recycling its pv slots
                    if p > 0:
                        oraw_evac(pv[p - 1])
                    pv[p] = pv_alloc(p)
                mult_step(p, c, e_tiles[g - LAG])
                pv_step(p, c, e_tiles[g - LAG], pv[p])

            fillers = {
                # v fillers start at g=2 so their wvt DMA wait never sits at
                # the head of the PE queue in front of the first logit steps
                2: lambda: v_proj(0),
                3: lambda: v_proj(1),
                4: lambda: (v_proj(2), kt_proj(1, 0)),
                5: lambda: (v_proj(3), v_proj(4), qt_proj(1)),
                6: lambda: (v_proj(5), v_proj(6)),
                7: lambda: v_proj(7),
                9: lambda: kt_proj(1, 1),
                11: lambda: kt_proj(2, 0),
                13: lambda: kt_proj(2, 1),
                14: lambda: qt_proj(2),
                19: lambda: kt_proj(3, 0),
                21: lambda: kt_proj(3, 1),
                22: lambda: qt_proj(3),
                # norm A (heads 0-3): oraw p0 lands in trail g=11, oraw p1 in
                # trail g=19 -> gather A at g=20; rbc heads a step after the
                # recip chain so the PE never waits on the DVE.
                20: lambda: state.update(denA=norm_gather(0, 4)),
                23: lambda: state.update(recipA=norm_recip(0, 4, state["denA"])),
                24: lambda: norm_head(0, 4, 0, state["recipA"]),
                25: lambda: norm_head(0, 4, 1, state["recipA"]),
                26: lambda: (norm_head(0, 4, 2, state["recipA"]), gate_proj(0)),
                27: lambda: (
                    norm_head(0, 4, 3, state["recipA"]),
                    norm_reloc(0, 4),
                ),
                # oraw p2 lands at trail g=27; B1 = heads 4-5
                28: lambda: (state.update(denB1=norm_gather(4, 2)), gate_proj(1)),
                29: lambda: state.update(recipB1=norm_recip(4, 2, state["denB1"])),
                30: lambda: gate_proj(2),
                31: lambda: (
                    norm_head(4, 2, 0, state["recipB1"]),
                    norm_head(4, 2, 1, state["recipB1"]),
                    norm_reloc(4, 2),
                ),
                # lead stream is done after g=31; to_out partials ko0-2 fill
                # the trail-only steps (ps_s slots now free of s-tiles)
                32: lambda: (toout_partial(0, (0, 1, 2)), toout_partial(1, (0, 1, 2))),
                33: lambda: (toout_partial(2, (0, 1, 2)), toout_partial(3, (0, 1, 2))),
            }

            for g in range(8 * 4 + LAG):
                lead(g)
                if g >= LAG:
                    trail(g)
                f = fillers.get(g)
                if f is not None:
                    f()

            # ---- tail ----
            # oraw p3 + den gather emitted BEFORE gate3's tanh so the ACT
            # queue doesn't delay the norm-B2 chain; gate3 follows.
            oraw_evac(pv[3])
            denB2 = norm_gather(6, 2)
            gate_proj(3)
            recipB2 = norm_recip(6, 2, denB2)
            norm_head(6, 2, 0, recipB2)  # h6 -> ofin[0:64, 3] direct
            norm_head(6, 2, 1, recipB2)  # h7 -> osc[:, 7]
            # no reloc for h7: to_out's ko3 splits into two K=64 matmuls --
            # the odd half reads osc directly against host-relocated wot2
            # (cuts ~2us of SBUF-relocation DMA latency off the tail)
            for mo in range(4):
                ms = slice(mo * P, (mo + 1) * P)
                nc.tensor.matmul(
                    toout_slot(mo),
                    lhsT=wot_sb[0:D, 3, ms],
                    rhs=ofin_sb[0:D, 3, :],
                    start=False,
                    stop=False,
                )
                nc.tensor.matmul(
                    toout_slot(mo),
                    lhsT=wot2_sb[:, ms],
                    rhs=osc_sb[:, 7, :],
                    start=False,
                    stop=True,
                )
                toout_finish(mo)

    nc.compile()
    return nc


def make_in_maps(q_x, attn_bias, Wq, bq, Wk, Wv, Wo, bo, Wg, bg, gating_bias):
    import ml_dtypes

    bf16 = ml_dtypes.bfloat16
    fp8 = ml_dtypes.float8_e4m3
    scale = np.float32(D) ** -0.5

    def swz(a2d):
        """[512, M] -> [128, 4, M] SBUF layout (partition-inner on dim 0)."""
        m = a2d.shape[1]
        return np.ascontiguousarray(a2d.reshape(4, P, m).transpose(1, 0, 2))

    # gate weights in fp8 scaled x32 (the tanh evac divides it back out);
    # q/k stay bf16 -- fp8 there costs ~1.4% output error (softmax-weight
    # noise does not average down)
    wqt = swz(Wq.T.astype(np.float32) * scale).astype(bf16)
    wkt = swz(np.asarray(Wk.T, dtype=np.float32)).astype(bf16)
    wgt = (swz(np.asarray(Wg.T, np.float32)) * 32.0).astype(fp8)
    wvt = swz(np.asarray(Wv.T, dtype=np.float32)).astype(bf16)
    wot = swz(np.asarray(Wo.T, dtype=np.float32)).astype(bf16)
    # h7's Wo rows staged at partitions 0-63 (reloc-free ko3-odd matmul)
    wot2 = np.ascontiguousarray(np.asarray(Wo.T, np.float32)[7 * D : 8 * D, :]).astype(
        bf16
    )
    bqs = np.ascontiguousarray((bq * scale).reshape(4, P).T).astype(np.float32)
    bo_ = np.ascontiguousarray(np.asarray(bo).reshape(4, P).T).astype(np.float32)
    gb = np.ascontiguousarray(
        (0.5 * (bg + gating_bias)).reshape(4, P).T
    ).astype(np.float32)
    sel8 = np.repeat(np.eye(H, dtype=np.float32), D, axis=1).astype(bf16)

    in_maps = []
    for c in range(8):
        b, half = c // 2, c % 2
        o0, o1 = half * NQ, (1 - half) * NQ
        x = np.asarray(q_x[b], dtype=np.float32)  # [N, CQ]
        # keys permuted: own query-row half first (q reads prefix of xt)
        xp = np.concatenate([x[o0 : o0 + NQ], x[o1 : o1 + NQ]], axis=0)
        xts = swz(np.ascontiguousarray(xp.T))
        xt = xts.astype(bf16)  # [128, 4, N]
        xt8 = np.ascontiguousarray(xts[:, :, 0:NQ]).astype(fp8)  # own rows
        # ebt[p, pair*16 + c*2 + h01, q] = exp(bias)[2*pair+h01,
        #   perm_key[c*128+p], own_row q]
        eb = np.exp(np.asarray(attn_bias[b, :, o0 : o0 + NQ, :], np.float32))
        ebp = np.concatenate(
            [eb[:, :, o0 : o0 + NQ], eb[:, :, o1 : o1 + NQ]], axis=2
        )  # [H, q, k(perm)]
        ebt = (
            ebp.transpose(0, 2, 1)  # [H, k, q]
            .reshape(4, 2, 8, P, NQ)  # [pair, h01, c, p, q]
            .transpose(3, 0, 2, 1, 4)  # [p, pair, c, h01, q]
            .reshape(P, H * 8, NQ)
        )
        ebt = np.ascontiguousarray(ebt).astype(bf16)
        in_maps.append(
            {
                "xt": xt,
                "xt8": xt8,
                "ebt": ebt,
                "wqt": wqt,
                "wkt": wkt,
                "wvt": wvt,
                "wot": wot,
                "wot2": wot2,
                "wgt": wgt,
                "bqs": bqs,
                "bo": bo_,
                "gb": gb,
                "sel8": sel8,
            }
        )
    return in_maps


_NC_CACHE = None


def kernel(**inputs) -> np.ndarray:
    global _NC_CACHE
    from concourse.bass_utils import run_bass_kernel_spmd

    if _NC_CACHE is None:
        _NC_CACHE = build_nc()
    nc = _NC_CACHE
    in_maps = make_in_maps(**inputs)
    trace = bool(int(os.environ.get("BASS_KERNEL_TRACE", "0")))
    last_exc = None
    for attempt in range(3):
        try:
            res = run_bass_kernel_spmd(nc, in_maps, list(range(8)), trace=trace)
            break
        except Exception as exc:  # transient NRT/axon device hiccups
            last_exc = exc
            import time

            time.sleep(10 * (attempt + 1))
    else:
        raise last_exc
    kernel.last_result = res
    out = np.empty((B, N, CQ), dtype=np.float32)
    for c in range(8):
        b, half = c // 2, c % 2
        # res "out" is [128, 4, NQ]: out^T[cq=o*128+i, q] at [i, o, q]
        o = res.results[c]["out"]
        out[b, half * NQ : (half + 1) * NQ, :] = (
            o.transpose(1, 0, 2).reshape(CQ, NQ).T.astype(np.float32)
        )
    return out
